# revision 27
# baseline (speedup 1.0000x reference)
"""Trainium2 Bass kernel for nn_CubicModelLarge (3-layer cubic-feature MLP).

Tensor-parallel over the cubic multiplier index i (64 values, 8 per core).
The cubic expansion is never materialized.  Per layer:

  y[b,o] = W_lin@x + b + sum_t W_sq[o,t] xsq[b,t] + sum_i x[b,i] sum_t W_cu[o,i,t] xsq[b,t]

Per core c (i in I_c = [8c, 8c+8)):

  H[b,(il,o)] = sum_J F[J,b] * Wcub[J,(il,o)]     (fp16 GEMM, J = 17x128 rows)
  y_c[b,o]    = lin[b,o] + b + sum_il x[b, i(il)] * H[b,(il,o)]
  y = AllReduce_c(y_c)

F chunks (128 rows each, per half-batch of 512):
  k=0..15 : [rot(2k+1); rot(2k+2)] products  x_a*x_{(a+d)%64}, d = 2k+1 + p//64
  k=16    : [x rows (carrying the symmetrized W_sq fold); squares x_a^2]
Rotated copies are built with PE selection matmuls -> ACT copy to fp16 SBUF ->
DVE 2x-mode products.  x^T itself (X2 = [x^T; x^T]) comes from a single
xbar transpose-DMA of the column-duplicated AllReduce payload (512, 128) --
no PE transposes anywhere.  The GEMM runs chunk-outer across 4 batch-chunk
PSUM banks so PE stays dense; per-sample combine is DVE scalar_tensor_tensor
with x slices taken batch-major (no selection matmuls for xmac).
Final-layer partials are summed on the host in fp32.
"""

import numpy as np

F16NP = np.float16

D = 64
B = 1024
NCORES = 8
I_PER = D // NCORES          # 8
OUTS = (64, 64, 10)
NK = 16                      # rotation-pair chunks
HB = 512                     # half-batch
NBH = HB // 128              # 4 batch chunks per half
PIPE = 2                     # rep-matmul software pipeline depth
FILL_N = 0                   # boundary filler matmuls (keep HAM warm)

_CACHE = {}


# ---------------------------------------------------------------- host prep --

def _maps():
    iu, ju = np.triu_indices(D)
    tmap = np.zeros((D, D), np.int64)
    tmap[iu, ju] = np.arange(len(iu))
    tmap[ju, iu] = tmap[iu, ju]
    p = np.arange(128)
    rows_t = np.zeros((NK, 128), np.int64)
    for k in range(NK):
        d = 2 * k + 1 + p // 64
        a = p % 64
        rows_t[k] = tmap[a, (a + d) % D]
    diag_t = tmap[np.arange(D), np.arange(D)]
    return tmap, rows_t, diag_t


def _prep_layer(W, b, out):
    """-> wcub [NCORES](17*128, I_PER*out) fp16, wlin [NCORES](64, out) fp16"""
    _, rows_t, diag_t = _maps()
    W_lin = W[:, :D]
    W_sq = W[:, D:D + 2080]
    W_cu = W[:, D + 2080:].reshape(out, D, 2080)

    iu, ju = np.triu_indices(D)
    w2 = np.zeros((out, D, D), np.float32)
    half = np.where(iu == ju, 1.0, 0.5).astype(np.float32)
    w2[:, iu, ju] = W_sq * half
    w2[:, ju, iu] = W_sq * half

    rt = rows_t.reshape(-1)
    # gap-32 rows (upper half of chunk 15) are double-counted -> halve
    scale = np.ones(NK * 128, np.float32)
    scale[15 * 128 + 64:16 * 128] = 0.5

    wcubs, wlins = [], []
    for core in range(NCORES):
        I = np.arange(core * I_PER, (core + 1) * I_PER)
        M = I_PER * out
        # column order m = o * I_PER + il so the combine can reduce over
        # il as the innermost (stride-1) axis
        wcub = np.zeros((17 * 128, M), np.float32)
        blk = W_cu[:, I, :][:, :, rt] * scale[None, None, :]
        wcub[:NK * 128] = blk.transpose(2, 0, 1).reshape(NK * 128, M)
        w2blk = w2[:, I, :]                                 # (out, I_PER, 64)
        wcub[NK * 128:NK * 128 + D] = w2blk.transpose(2, 0, 1).reshape(D, M)
        dblk = W_cu[:, I, :][:, :, diag_t]
        wcub[NK * 128 + D:] = dblk.transpose(2, 0, 1).reshape(D, M)
        wcubs.append(np.ascontiguousarray(wcub.astype(F16NP)))

        wl = np.zeros((D, out), np.float32)
        if core == 0:
            wl[:] = W_lin.T
        wlins.append(np.ascontiguousarray(wl.astype(F16NP)))
    return wcubs, wlins


def _sel_consts():
    """PE selection matrices (64, NK*128): slot k -> [rot(2k+1); rot(2k+2)]."""
    sel = np.zeros((D, NK * 128), np.float32)
    for k in range(NK):
        for p in range(128):
            d = 2 * k + 1 + p // 64
            a = p % 64
            sel[(a + d) % D, k * 128 + p] = 1.0
    return sel


# ------------------------------------------------------------------ builder --

def _build_module():
    import concourse.bacc as bacc
    import concourse.mybir as mybir
    import concourse.tile as tile

    F32 = mybir.dt.float32
    F16 = mybir.dt.float16
    MULT = mybir.AluOpType.mult
    ADD = mybir.AluOpType.add

    nc = bacc.Bacc("TRN2", target_bir_lowering=False, num_devices=NCORES, debug=False)

    x16_in = nc.dram_tensor("x16", [B, D], F16, kind="ExternalInput")
    wcub_in = [
        nc.dram_tensor(f"wcub{li}", [17 * 128, I_PER * OUTS[li]], F16, kind="ExternalInput")
        for li in range(3)
    ]
    wlin_in = [
        nc.dram_tensor(f"wlin{li}", [D, OUTS[li]], F16, kind="ExternalInput")
        for li in range(3)
    ]
    # per-core smalls packed in one tensor: colsel | bfull0 | bfull1 | bfull2
    SM_COLS = I_PER + NBH * (OUTS[0] + OUTS[1] + OUTS[2])
    smalls_in = nc.dram_tensor("smalls", [128, SM_COLS], F16, kind="ExternalInput")
    out_ext = nc.dram_tensor("out", [B, OUTS[2]], F32, kind="ExternalOutput")

    constc_np = np.zeros((128, NK * 128 + 128), np.float16)
    constc_np[:D, :NK * 128] = _sel_consts().astype(np.float16)
    constc_np[:, NK * 128:] = np.eye(128, dtype=np.float16)
    const_c = nc.inline_tensor(constc_np, name="constc")

    with tile.TileContext(nc) as tc:
        with (
            tc.tile_pool(name="spool", bufs=1) as spool,
            tc.tile_pool(name="wpool", bufs=3) as wpool,
            tc.tile_pool(name="xpool", bufs=2) as xpool,
            tc.tile_pool(name="fpool", bufs=4) as fpool,
            tc.tile_pool(name="qpool", bufs=3) as qpool,
            tc.tile_pool(name="hpool", bufs=2) as hpool,
            tc.tile_pool(name="ypool", bufs=2) as ypool,
            tc.tile_pool(name="ps_rep", bufs=2, space="PSUM") as ps_rep,
            tc.tile_pool(name="ps_h", bufs=1, space="PSUM") as ps_h,
            tc.tile_pool(name="ps_small", bufs=2, space="PSUM") as ps_small,
            tc.tile_pool(name="dpool", bufs=2, space="DRAM") as dpool,
        ):
            # layer-0 activations: batch-major, first on the sync queue
            xs_t = [None, None]
            pending_xs = [None]
            for h in range(2):
                xs0 = xpool.tile([128, NBH, D], F16, tag=f"xsh{h}")
                nc.sync.dma_start(
                    xs0[:],
                    x16_in.ap()[h * HB:(h + 1) * HB, :]
                    .rearrange("(bc p) f -> p bc f", p=128),
                )
                xs_t[h] = xs0

            constc_sb = spool.tile([128, NK * 128 + 128], F16, tag="constc")
            nc.scalar.dma_start(constc_sb[:], const_c.ap())
            sel_sb = constc_sb[0:D, 0:NK * 128]
            ident16_sb = constc_sb[:, NK * 128:]
            smalls_sb = spool.tile([128, SM_COLS], F16, tag="smalls")
            nc.scalar.dma_start(smalls_sb[:], smalls_in.ap())
            colsel_sb = smalls_sb[0:D, 0:I_PER]
            off = I_PER
            bfull_sb = []
            for li in range(3):
                bfull_sb.append(
                    smalls_sb[:, off:off + NBH * OUTS[li]]
                    .rearrange("p (bc o) -> p bc o", bc=NBH))
                off += NBH * OUTS[li]

            # warm up the collectives path with a tiny dummy AllReduce that
            # overlaps layer-0 compute
            warm_in = dpool.tile([16, 16], F16, tag="warmin")
            warm_out = dpool.tile([16, 16], F16, tag="warmout")
            nc.gpsimd.collective_compute(
                "AllReduce",
                ADD,
                replica_groups=[list(range(NCORES))],
                ins=[warm_in.opt()],
                outs=[warm_out.opt()],
            )

            weights = []
            for li in range(3):
                M = I_PER * OUTS[li]
                wcub_sb = wpool.tile([128, 17, M], F16, tag="wcub", name=f"wcub_sb{li}")
                wlin_sb = wpool.tile([D, OUTS[li]], F16, tag="wlin", name=f"wlin_sb{li}")
                weights.append((wcub_sb, wlin_sb))

            def dma_weights(li, split=False):
                wcub_sb, wlin_sb = weights[li]
                src = wcub_in[li].ap().rearrange("(k p) m -> p k m", p=128)
                if split:
                    nc.scalar.dma_start(wcub_sb[:, 0:3, :], src[:, 0:3, :])
                    nc.scalar.dma_start(wcub_sb[:, 3:9, :], src[:, 3:9, :])
                    # bulk of layer-0 on the (otherwise idle) sync queue
                    nc.sync.dma_start(wcub_sb[:, 9:17, :], src[:, 9:17, :])
                    nc.scalar.dma_start(wlin_sb[:], wlin_in[li].ap())
                    return
                nc.scalar.dma_start(wcub_sb[:, 0:9, :], src[:, 0:9, :])
                nc.scalar.dma_start(wcub_sb[:, 9:17, :], src[:, 9:17, :])
                nc.scalar.dma_start(wlin_sb[:], wlin_in[li].ap())

            dma_weights(0, split=True)
            dma_weights(1)
            dma_weights(2)

            for li in range(3):
                out_l = OUTS[li]
                M = I_PER * out_l
                last = li == 2
                wcub_sb, wlin_sb = weights[li]

                for h in range(2):
                    # flush the previous boundary's deferred activation-return
                    # DMA (kept off the front of the sync queue so it never
                    # blocks the next bounce write behind an AllReduce)
                    if pending_xs[0] is not None:
                        pxs, pred = pending_xs[0]
                        nc.sync.dma_start(
                            pxs[:],
                            pred[:].rearrange("(bc p) f -> p bc f", p=128),
                        )
                        pending_xs[0] = None
                    # rebuild x2 = [x^T; x^T] from the batch-major activations
                    # with PE transposes (in the consumer body so the PE queue
                    # never head-of-line blocks on the AllReduce)
                    xs = xs_t[h]
                    x2 = xpool.tile([128, HB], F16, tag=f"x2h{h}")
                    for bc in range(NBH):
                        bs = slice(bc * 128, (bc + 1) * 128)
                        tp = ps_rep.tile([D, 128], F16, tag="rep")
                        nc.tensor.transpose(tp[:], xs[:, bc, :], ident16_sb[:])
                        nc.scalar.copy(x2[0:D, bs], tp[:])
                    nc.scalar.copy(x2[D:128, :], x2[0:D, :])

                    lin_ps = ps_small.tile(
                        [128, NBH, out_l + I_PER], F32, tag="lin")
                    for bc in range(NBH):
                        bs = slice(bc * 128, (bc + 1) * 128)
                        nc.tensor.matmul(lin_ps[:, bc, out_l:], x2[0:D, bs],
                                         colsel_sb[:], start=True, stop=True)
                    xmac = xpool.tile([128, NBH, I_PER], F16, tag=f"xmh{h}")
                    nc.scalar.copy(xmac[:], lin_ps[:, :, out_l:])

                    # rep matmul -> product, software-pipelined against the
                    # chunk-outer GEMM; products alternate ACT-copy+2x-DVE and
                    # direct-PSUM 1x-DVE to balance the two engines
                    if not last:
                        hps = [ps_h.tile([128, M], F32, tag=f"h{bc}", name=f"hps{bc}")
                               for bc in range(NBH)]
                    else:
                        h2_ps = ps_h.tile([M, HB], F32, tag="h0")

                    fks = [None] * 17

                    def make_fk(k):
                        if k < NK:
                            rep = ps_rep.tile([128, HB], F32, tag="rep")
                            nc.tensor.matmul(
                                rep[:], sel_sb[:, k * 128:(k + 1) * 128],
                                x2[0:D, :], start=True, stop=True,
                            )
                            fk = fpool.tile([128, HB], F16, tag="f")
                            if k % 2 == 0:
                                rk = qpool.tile([128, HB], F16, tag="rk")
                                nc.scalar.copy(rk[:], rep[:])
                                nc.vector.tensor_mul(fk[:], x2[:], rk[:])
                            else:
                                nc.vector.tensor_mul(fk[:], x2[:], rep[:])
                        else:
                            fk = fpool.tile([128, HB], F16, tag="f")
                            nc.scalar.copy(fk[0:D, :], x2[0:D, :])
                            nc.vector.tensor_mul(fk[D:128, :], x2[D:128, :], x2[D:128, :])
                        fks[k] = fk

                    def consume_fk(k):
                        fk = fks[k]
                        first, last_k = k == 0, k == 16
                        if not last:
                            for bc in range(NBH):
                                bs = slice(bc * 128, (bc + 1) * 128)
                                nc.tensor.matmul(
                                    hps[bc][:], fk[:, bs], wcub_sb[:, k, :],
                                    start=first, stop=last_k,
                                )
                        else:
                            nc.tensor.matmul(
                                h2_ps[:], wcub_sb[:, k, :], fk[:],
                                start=first, stop=last_k,
                            )

                    for k in range(17 + PIPE):
                        if k < 17:
                            make_fk(k)
                        if k >= PIPE:
                            consume_fk(k - PIPE)

                    # linear part + bias -> y_base
                    for bc in range(NBH):
                        bs = slice(bc * 128, (bc + 1) * 128)
                        nc.tensor.matmul(lin_ps[:, bc, 0:out_l], x2[0:D, bs],
                                         wlin_sb[:], start=True, stop=True)

                    ybase = ypool.tile([128, NBH, out_l], F16, tag="yb")
                    nc.vector.tensor_add(ybase[:], lin_ps[:, :, 0:out_l],
                                         bfull_sb[li][:])

                    y_sb = ypool.tile([128, NBH, out_l], F32 if last else F16,
                                      tag=f"y{'2' if last else ''}")

                    if not last:
                        # combine: P = H * xmac (broadcast over o, il innermost),
                        # reduce over il, add lin+bias
                        hsb = hpool.tile([128, NBH, M], F16, tag="hs")
                        for bc in range(NBH):
                            nc.scalar.copy(hsb[:, bc, :], hps[bc][:])
                        psc = hpool.tile([128, NBH, out_l, I_PER], F16, tag="p")
                        nc.vector.tensor_mul(
                            psc[:],
                            hsb[:].rearrange("p bc (o il) -> p bc o il", il=I_PER),
                            xmac[:].unsqueeze(2).broadcast_to(
                                (128, NBH, out_l, I_PER)),
                        )
                        red_t = ypool.tile([128, NBH, out_l], F32, tag="red")
                        nc.vector.tensor_reduce(
                            red_t[:], psc[:], axis=mybir.AxisListType.X,
                            op=ADD)
                        nc.vector.tensor_add(y_sb[:], red_t[:], ybase[:])

                        # per-half AllReduce; batch-major return DMA deferred
                        bounce = dpool.tile([HB, D], F16, tag=f"bounce{h}")
                        red = dpool.tile([HB, D], F16, tag=f"red{h}")
                        nc.sync.dma_start(
                            bounce[:].rearrange("(bc p) o -> p bc o", p=128),
                            y_sb[:],
                        )
                        nc.gpsimd.collective_compute(
                            "AllReduce",
                            ADD,
                            replica_groups=[list(range(NCORES))],
                            ins=[bounce.opt()],
                            outs=[red.opt()],
                        )
                        xs_n = xpool.tile([128, NBH, D], F16, tag=f"xsh{h}")
                        pending_xs[0] = (xs_n, red)
                        xs_t[h] = xs_n
                        # keep the PE busy (HAM warm) while the AllReduce
                        # flies -- results are never read
                        for fi in range(FILL_N):
                            frep = ps_rep.tile([128, HB], F32, tag="rep",
                                               name=f"fill{li}{h}_{fi}")
                            nc.tensor.matmul(
                                frep[:], sel_sb[:, 0:128], x2[0:D, :],
                                start=True, stop=True,
                            )
                    else:
                        h2_sb = hpool.tile([M, HB], F16, tag="hs")
                        nc.scalar.copy(h2_sb[:], h2_ps[:])
                        for bc in range(NBH):
                            bs = slice(bc * 128, (bc + 1) * 128)
                            t_ps = ps_rep.tile([128, M], F16, tag="rep")
                            nc.tensor.transpose(t_ps[:], h2_sb[:, bs],
                                                ident16_sb[0:M, 0:M])
                            h2c = qpool.tile([128, M], F16, tag="rk")
                            nc.scalar.copy(h2c[:], t_ps[:])
                            p2 = ypool.tile([128, out_l, I_PER], F16, tag="p2")
                            nc.vector.tensor_mul(
                                p2[:],
                                h2c[:].rearrange("p (o il) -> p o il", il=I_PER),
                                xmac[:, bc, :].unsqueeze(1).broadcast_to(
                                    (128, out_l, I_PER)),
                            )
                            r2 = ypool.tile([128, out_l], F32, tag="r2")
                            nc.vector.tensor_reduce(
                                r2[:], p2[:], axis=mybir.AxisListType.X, op=ADD)
                            nc.vector.tensor_add(y_sb[:, bc, :], r2[:],
                                                 ybase[:, bc, :])
                        nc.sync.dma_start(
                            out_ext.ap()[h * HB:(h + 1) * HB, :]
                            .rearrange("(bc p) o -> p bc o", p=128),
                            y_sb[:],
                        )

    nc.compile()
    return nc


# ------------------------------------------------------------------- runner --

def build_in_maps(x, W0, b0, W1, b1, W2, b2):
    x = np.asarray(x, np.float32)
    Ws = [np.asarray(W, np.float32) for W in (W0, W1, W2)]
    bs = [np.asarray(b_, np.float32) for b_ in (b0, b1, b2)]

    x16 = np.ascontiguousarray(x.astype(F16NP))

    wcubs, wlins = {}, {}
    for li in range(3):
        wcubs[li], wlins[li] = _prep_layer(Ws[li], bs[li], OUTS[li])

    in_maps = []
    for core in range(NCORES):
        I = np.arange(core * I_PER, (core + 1) * I_PER)
        sm_cols = I_PER + NBH * (OUTS[0] + OUTS[1] + OUTS[2])
        smalls = np.zeros((128, sm_cols), np.float32)
        smalls[I, np.arange(I_PER)] = 1.0
        off = I_PER
        for li in range(3):
            if core == 0:
                smalls[:, off:off + NBH * OUTS[li]] = np.tile(
                    bs[li], (128, NBH, 1)).reshape(128, NBH * OUTS[li])
            off += NBH * OUTS[li]
        m = {"x16": x16, "smalls": smalls.astype(F16NP)}
        for li in range(3):
            m[f"wcub{li}"] = wcubs[li][core]
            m[f"wlin{li}"] = wlins[li][core]
        in_maps.append(m)
    return in_maps


def kernel(x, W0, b0, W1, b1, W2, b2):
    from concourse.bass_utils import run_bass_kernel_spmd

    if "nc" not in _CACHE:
        _CACHE["nc"] = _build_module()
    nc = _CACHE["nc"]

    in_maps = build_in_maps(x, W0, b0, W1, b1, W2, b2)
    res = run_bass_kernel_spmd(nc, in_maps, core_ids=list(range(NCORES)))
    out = np.zeros((B, OUTS[2]), np.float32)
    for core in range(NCORES):
        out += res.results[core]["out"]
    return out


# revision 28
# speedup vs baseline: 1.1072x; 1.1072x over previous
"""Trainium2 Bass kernel for nn_CubicModelLarge (3-layer cubic-feature MLP).

Tensor-parallel over the cubic multiplier index i (64 values, 8 per core).
The cubic expansion is never materialized.  Per layer:

  y[b,o] = W_lin@x + b + sum_t W_sq[o,t] xsq[b,t] + sum_i x[b,i] sum_t W_cu[o,i,t] xsq[b,t]

Per core c (i in I_c = [8c, 8c+8)):

  H[b,(il,o)] = sum_J F[J,b] * Wcub[J,(il,o)]     (fp16 GEMM, J = 17x128 rows)
  y_c[b,o]    = lin[b,o] + b + sum_il x[b, i(il)] * H[b,(il,o)]
  y = AllReduce_c(y_c)

F chunks (128 rows each, per half-batch of 512):
  k=0..15 : [rot(2k+1); rot(2k+2)] products  x_a*x_{(a+d)%64}, d = 2k+1 + p//64
  k=16    : [x rows (carrying the symmetrized W_sq fold); squares x_a^2]
Rotated copies are built with PE selection matmuls -> ACT copy to fp16 SBUF ->
DVE 2x-mode products.  x^T itself (X2 = [x^T; x^T]) comes from a single
xbar transpose-DMA of the column-duplicated AllReduce payload (512, 128) --
no PE transposes anywhere.  The GEMM runs chunk-outer across 4 batch-chunk
PSUM banks so PE stays dense; per-sample combine is DVE scalar_tensor_tensor
with x slices taken batch-major (no selection matmuls for xmac).
Final-layer partials are summed on the host in fp32.
"""

import numpy as np

F16NP = np.float16

D = 64
B = 1024
NCORES = 8
I_PER = D // NCORES          # 8
OUTS = (64, 64, 10)
NK = 16                      # rotation-pair chunks
HB = 512                     # half-batch
NBH = HB // 128              # 4 batch chunks per half
PIPE = 2                     # rep-matmul software pipeline depth
FILL_N = 0                   # boundary filler matmuls (keep HAM warm)

_CACHE = {}


# ---------------------------------------------------------------- host prep --

def _maps():
    iu, ju = np.triu_indices(D)
    tmap = np.zeros((D, D), np.int64)
    tmap[iu, ju] = np.arange(len(iu))
    tmap[ju, iu] = tmap[iu, ju]
    p = np.arange(128)
    rows_t = np.zeros((NK, 128), np.int64)
    for k in range(NK):
        d = 2 * k + 1 + p // 64
        a = p % 64
        rows_t[k] = tmap[a, (a + d) % D]
    diag_t = tmap[np.arange(D), np.arange(D)]
    return tmap, rows_t, diag_t


def _prep_layer(W, b, out):
    """-> wcub [NCORES](17*128, I_PER*out) fp16, wlin [NCORES](64, out) fp16"""
    _, rows_t, diag_t = _maps()
    W_lin = W[:, :D]
    W_sq = W[:, D:D + 2080]
    W_cu = W[:, D + 2080:].reshape(out, D, 2080)

    iu, ju = np.triu_indices(D)
    w2 = np.zeros((out, D, D), np.float32)
    half = np.where(iu == ju, 1.0, 0.5).astype(np.float32)
    w2[:, iu, ju] = W_sq * half
    w2[:, ju, iu] = W_sq * half

    rt = rows_t.reshape(-1)
    # gap-32 rows (upper half of chunk 15) are double-counted -> halve
    scale = np.ones(NK * 128, np.float32)
    scale[15 * 128 + 64:16 * 128] = 0.5

    wcubs, wlins = [], []
    for core in range(NCORES):
        I = np.arange(core * I_PER, (core + 1) * I_PER)
        M = I_PER * out
        # column order m = o * I_PER + il so the combine can reduce over
        # il as the innermost (stride-1) axis
        wcub = np.zeros((17 * 128, M), np.float32)
        blk = W_cu[:, I, :][:, :, rt] * scale[None, None, :]
        wcub[:NK * 128] = blk.transpose(2, 0, 1).reshape(NK * 128, M)
        w2blk = w2[:, I, :]                                 # (out, I_PER, 64)
        wcub[NK * 128:NK * 128 + D] = w2blk.transpose(2, 0, 1).reshape(D, M)
        dblk = W_cu[:, I, :][:, :, diag_t]
        wcub[NK * 128 + D:] = dblk.transpose(2, 0, 1).reshape(D, M)
        wcubs.append(np.ascontiguousarray(wcub.astype(F16NP)))

        wl = np.zeros((D, out), np.float32)
        if core == 0:
            wl[:] = W_lin.T
        wlins.append(np.ascontiguousarray(wl.astype(F16NP)))
    return wcubs, wlins


def _sel_consts():
    """PE selection matrices (64, NK*128): slot k -> [rot(2k+1); rot(2k+2)]."""
    sel = np.zeros((D, NK * 128), np.float32)
    for k in range(NK):
        for p in range(128):
            d = 2 * k + 1 + p // 64
            a = p % 64
            sel[(a + d) % D, k * 128 + p] = 1.0
    return sel


# ------------------------------------------------------------------ builder --

def _build_module():
    import concourse.bacc as bacc
    import concourse.mybir as mybir
    import concourse.tile as tile

    F32 = mybir.dt.float32
    F16 = mybir.dt.float16
    MULT = mybir.AluOpType.mult
    ADD = mybir.AluOpType.add

    nc = bacc.Bacc("TRN2", target_bir_lowering=False, num_devices=NCORES, debug=False)

    x16_in = nc.dram_tensor("x16", [B, D], F16, kind="ExternalInput")
    wcub_in = [
        nc.dram_tensor(f"wcub{li}", [17 * 128, I_PER * OUTS[li]], F16, kind="ExternalInput")
        for li in range(3)
    ]
    wlin_in = [
        nc.dram_tensor(f"wlin{li}", [D, OUTS[li]], F16, kind="ExternalInput")
        for li in range(3)
    ]
    # per-core smalls packed in one tensor: colsel | bfull0 | bfull1 | bfull2
    SM_COLS = I_PER + NBH * (OUTS[0] + OUTS[1] + OUTS[2])
    smalls_in = nc.dram_tensor("smalls", [128, SM_COLS], F16, kind="ExternalInput")
    out_ext = nc.dram_tensor("out", [B, OUTS[2]], F32, kind="ExternalOutput")

    constc_np = np.zeros((128, NK * 128 + 128), np.float16)
    constc_np[:D, :NK * 128] = _sel_consts().astype(np.float16)
    constc_np[:, NK * 128:] = np.eye(128, dtype=np.float16)
    const_c = nc.inline_tensor(constc_np, name="constc")

    with tile.TileContext(nc) as tc:
        with (
            tc.tile_pool(name="spool", bufs=1) as spool,
            tc.tile_pool(name="wpool", bufs=3) as wpool,
            tc.tile_pool(name="xpool", bufs=2) as xpool,
            tc.tile_pool(name="fpool", bufs=4) as fpool,
            tc.tile_pool(name="qpool", bufs=3) as qpool,
            tc.tile_pool(name="hpool", bufs=2) as hpool,
            tc.tile_pool(name="ypool", bufs=2) as ypool,
            tc.tile_pool(name="ps_rep", bufs=2, space="PSUM") as ps_rep,
            tc.tile_pool(name="ps_h", bufs=1, space="PSUM") as ps_h,
            tc.tile_pool(name="ps_small", bufs=2, space="PSUM") as ps_small,
            tc.tile_pool(name="dpool", bufs=2, space="DRAM") as dpool,
        ):
            # layer-0 activations: batch-major, first on the sync queue
            xs_t = [None, None]
            pending_xs = [None]
            for h in range(2):
                xs0 = xpool.tile([128, NBH, D], F16, tag=f"xsh{h}")
                nc.sync.dma_start(
                    xs0[:],
                    x16_in.ap()[h * HB:(h + 1) * HB, :]
                    .rearrange("(bc p) f -> p bc f", p=128),
                )
                xs_t[h] = xs0

            constc_sb = spool.tile([128, NK * 128 + 128], F16, tag="constc")
            nc.scalar.dma_start(constc_sb[:], const_c.ap())
            sel_sb = constc_sb[0:D, 0:NK * 128]
            ident16_sb = constc_sb[:, NK * 128:]
            smalls_sb = spool.tile([128, SM_COLS], F16, tag="smalls")
            nc.sync.dma_start(smalls_sb[:], smalls_in.ap())
            colsel_sb = smalls_sb[0:D, 0:I_PER]
            off = I_PER
            bfull_sb = []
            for li in range(3):
                bfull_sb.append(
                    smalls_sb[:, off:off + NBH * OUTS[li]]
                    .rearrange("p (bc o) -> p bc o", bc=NBH))
                off += NBH * OUTS[li]

            weights = []
            for li in range(3):
                M = I_PER * OUTS[li]
                wcub_sb = wpool.tile([128, 17, M], F16, tag="wcub", name=f"wcub_sb{li}")
                wlin_sb = wpool.tile([D, OUTS[li]], F16, tag="wlin", name=f"wlin_sb{li}")
                weights.append((wcub_sb, wlin_sb))

            def dma_weights(li, split=False):
                wcub_sb, wlin_sb = weights[li]
                src = wcub_in[li].ap().rearrange("(k p) m -> p k m", p=128)
                if split:
                    nc.scalar.dma_start(wcub_sb[:, 0:2, :], src[:, 0:2, :])
                    nc.scalar.dma_start(wcub_sb[:, 2:7, :], src[:, 2:7, :])
                    nc.scalar.dma_start(wcub_sb[:, 7:12, :], src[:, 7:12, :])
                    # bulk of layer-0 on the (otherwise idle) sync queue
                    nc.sync.dma_start(wcub_sb[:, 12:17, :], src[:, 12:17, :])
                    nc.scalar.dma_start(wlin_sb[:], wlin_in[li].ap())
                    return
                nc.scalar.dma_start(wcub_sb[:, 0:9, :], src[:, 0:9, :])
                nc.scalar.dma_start(wcub_sb[:, 9:17, :], src[:, 9:17, :])
                nc.scalar.dma_start(wlin_sb[:], wlin_in[li].ap())

            dma_weights(0, split=True)
            dma_weights(1)
            dma_weights(2)

            for li in range(3):
                out_l = OUTS[li]
                M = I_PER * out_l
                last = li == 2
                wcub_sb, wlin_sb = weights[li]

                for h in range(2):
                    # flush the previous boundary's deferred activation-return
                    # DMA (kept off the front of the sync queue so it never
                    # blocks the next bounce write behind an AllReduce)
                    if pending_xs[0] is not None:
                        pxs, pred = pending_xs[0]
                        nc.sync.dma_start(
                            pxs[:],
                            pred[:].rearrange("(bc p) f -> p bc f", p=128),
                        )
                        pending_xs[0] = None
                    # rebuild x2 = [x^T; x^T] from the batch-major activations
                    # with PE transposes (in the consumer body so the PE queue
                    # never head-of-line blocks on the AllReduce)
                    xs = xs_t[h]
                    x2 = xpool.tile([128, HB], F16, tag=f"x2h{h}")
                    for bc in range(NBH):
                        bs = slice(bc * 128, (bc + 1) * 128)
                        tp = ps_rep.tile([D, 128], F16, tag="rep")
                        nc.tensor.transpose(tp[:], xs[:, bc, :], ident16_sb[:])
                        nc.scalar.copy(x2[0:D, bs], tp[:])
                    nc.scalar.copy(x2[D:128, :], x2[0:D, :])

                    lin_ps = ps_small.tile(
                        [128, NBH, out_l + I_PER], F32, tag="lin")
                    for bc in range(NBH):
                        bs = slice(bc * 128, (bc + 1) * 128)
                        nc.tensor.matmul(lin_ps[:, bc, out_l:], x2[0:D, bs],
                                         colsel_sb[:], start=True, stop=True)
                    xmac = xpool.tile([128, NBH, I_PER], F16, tag=f"xmh{h}")
                    nc.scalar.copy(xmac[:], lin_ps[:, :, out_l:])

                    # rep matmul -> product, software-pipelined against the
                    # chunk-outer GEMM; products alternate ACT-copy+2x-DVE and
                    # direct-PSUM 1x-DVE to balance the two engines
                    if not last:
                        hps = [ps_h.tile([128, M], F32, tag=f"h{bc}", name=f"hps{bc}")
                               for bc in range(NBH)]
                    else:
                        h2_ps = ps_h.tile([M, HB], F32, tag="h0")

                    fks = [None] * 17

                    def make_fk(k):
                        if k < NK:
                            rep = ps_rep.tile([128, HB], F32, tag="rep")
                            nc.tensor.matmul(
                                rep[:], sel_sb[:, k * 128:(k + 1) * 128],
                                x2[0:D, :], start=True, stop=True,
                            )
                            fk = fpool.tile([128, HB], F16, tag="f")
                            if k % 2 == 0:
                                rk = qpool.tile([128, HB], F16, tag="rk")
                                nc.scalar.copy(rk[:], rep[:])
                                nc.vector.tensor_mul(fk[:], x2[:], rk[:])
                            else:
                                nc.vector.tensor_mul(fk[:], x2[:], rep[:])
                        else:
                            fk = fpool.tile([128, HB], F16, tag="f")
                            nc.scalar.copy(fk[0:D, :], x2[0:D, :])
                            nc.vector.tensor_mul(fk[D:128, :], x2[D:128, :], x2[D:128, :])
                        fks[k] = fk

                    def consume_fk(k):
                        fk = fks[k]
                        first, last_k = k == 0, k == 16
                        if not last:
                            for bc in range(NBH):
                                bs = slice(bc * 128, (bc + 1) * 128)
                                nc.tensor.matmul(
                                    hps[bc][:], fk[:, bs], wcub_sb[:, k, :],
                                    start=first, stop=last_k,
                                )
                        else:
                            nc.tensor.matmul(
                                h2_ps[:], wcub_sb[:, k, :], fk[:],
                                start=first, stop=last_k,
                            )

                    for k in range(17 + PIPE):
                        if k < 17:
                            make_fk(k)
                        if k >= PIPE:
                            consume_fk(k - PIPE)

                    # linear part + bias -> y_base
                    for bc in range(NBH):
                        bs = slice(bc * 128, (bc + 1) * 128)
                        nc.tensor.matmul(lin_ps[:, bc, 0:out_l], x2[0:D, bs],
                                         wlin_sb[:], start=True, stop=True)

                    ybase = ypool.tile([128, NBH, out_l], F16, tag="yb")
                    nc.vector.tensor_add(ybase[:], lin_ps[:, :, 0:out_l],
                                         bfull_sb[li][:])

                    y_sb = ypool.tile([128, NBH, out_l], F32 if last else F16,
                                      tag=f"y{'2' if last else ''}")

                    if not last:
                        # combine: P = H * xmac (broadcast over o, il innermost),
                        # reduce over il, add lin+bias
                        hsb = hpool.tile([128, NBH, M], F16, tag="hs")
                        for bc in range(NBH):
                            nc.scalar.copy(hsb[:, bc, :], hps[bc][:])
                        psc = hpool.tile([128, NBH, out_l, I_PER], F16, tag="p")
                        nc.vector.tensor_mul(
                            psc[:],
                            hsb[:].rearrange("p bc (o il) -> p bc o il", il=I_PER),
                            xmac[:].unsqueeze(2).broadcast_to(
                                (128, NBH, out_l, I_PER)),
                        )
                        red_t = ypool.tile([128, NBH, out_l], F32, tag="red")
                        nc.vector.tensor_reduce(
                            red_t[:], psc[:], axis=mybir.AxisListType.X,
                            op=ADD)
                        nc.vector.tensor_add(y_sb[:], red_t[:], ybase[:])

                        # per-half AllReduce; batch-major return DMA deferred
                        bounce = dpool.tile([HB, D], F16, tag=f"bounce{h}")
                        red = dpool.tile([HB, D], F16, tag=f"red{h}")
                        nc.sync.dma_start(
                            bounce[:].rearrange("(bc p) o -> p bc o", p=128),
                            y_sb[:],
                        )
                        nc.gpsimd.collective_compute(
                            "AllReduce",
                            ADD,
                            replica_groups=[list(range(NCORES))],
                            ins=[bounce.opt()],
                            outs=[red.opt()],
                        )
                        xs_n = xpool.tile([128, NBH, D], F16, tag=f"xsh{h}")
                        pending_xs[0] = (xs_n, red)
                        xs_t[h] = xs_n
                        # keep the PE busy (HAM warm) while the AllReduce
                        # flies -- results are never read
                        for fi in range(FILL_N):
                            frep = ps_rep.tile([128, HB], F32, tag="rep",
                                               name=f"fill{li}{h}_{fi}")
                            nc.tensor.matmul(
                                frep[:], sel_sb[:, 0:128], x2[0:D, :],
                                start=True, stop=True,
                            )
                    else:
                        h2_sb = hpool.tile([M, HB], F16, tag="hs")
                        nc.scalar.copy(h2_sb[:], h2_ps[:])
                        for bc in range(NBH):
                            bs = slice(bc * 128, (bc + 1) * 128)
                            t_ps = ps_rep.tile([128, M], F16, tag="rep")
                            nc.tensor.transpose(t_ps[:], h2_sb[:, bs],
                                                ident16_sb[0:M, 0:M])
                            h2c = qpool.tile([128, M], F16, tag="rk")
                            nc.scalar.copy(h2c[:], t_ps[:])
                            p2 = ypool.tile([128, out_l, I_PER], F16, tag="p2")
                            nc.vector.tensor_mul(
                                p2[:],
                                h2c[:].rearrange("p (o il) -> p o il", il=I_PER),
                                xmac[:, bc, :].unsqueeze(1).broadcast_to(
                                    (128, out_l, I_PER)),
                            )
                            r2 = ypool.tile([128, out_l], F32, tag="r2")
                            nc.vector.tensor_reduce(
                                r2[:], p2[:], axis=mybir.AxisListType.X, op=ADD)
                            nc.vector.tensor_add(y_sb[:, bc, :], r2[:],
                                                 ybase[:, bc, :])
                        nc.sync.dma_start(
                            out_ext.ap()[h * HB:(h + 1) * HB, :]
                            .rearrange("(bc p) o -> p bc o", p=128),
                            y_sb[:],
                        )

    nc.compile()
    return nc


# ------------------------------------------------------------------- runner --

def build_in_maps(x, W0, b0, W1, b1, W2, b2):
    x = np.asarray(x, np.float32)
    Ws = [np.asarray(W, np.float32) for W in (W0, W1, W2)]
    bs = [np.asarray(b_, np.float32) for b_ in (b0, b1, b2)]

    x16 = np.ascontiguousarray(x.astype(F16NP))

    wcubs, wlins = {}, {}
    for li in range(3):
        wcubs[li], wlins[li] = _prep_layer(Ws[li], bs[li], OUTS[li])

    in_maps = []
    for core in range(NCORES):
        I = np.arange(core * I_PER, (core + 1) * I_PER)
        sm_cols = I_PER + NBH * (OUTS[0] + OUTS[1] + OUTS[2])
        smalls = np.zeros((128, sm_cols), np.float32)
        smalls[I, np.arange(I_PER)] = 1.0
        off = I_PER
        for li in range(3):
            if core == 0:
                smalls[:, off:off + NBH * OUTS[li]] = np.tile(
                    bs[li], (128, NBH, 1)).reshape(128, NBH * OUTS[li])
            off += NBH * OUTS[li]
        m = {"x16": x16, "smalls": smalls.astype(F16NP)}
        for li in range(3):
            m[f"wcub{li}"] = wcubs[li][core]
            m[f"wlin{li}"] = wlins[li][core]
        in_maps.append(m)
    return in_maps


def kernel(x, W0, b0, W1, b1, W2, b2):
    from concourse.bass_utils import run_bass_kernel_spmd

    if "nc" not in _CACHE:
        _CACHE["nc"] = _build_module()
    nc = _CACHE["nc"]

    in_maps = build_in_maps(x, W0, b0, W1, b1, W2, b2)
    res = run_bass_kernel_spmd(nc, in_maps, core_ids=list(range(NCORES)))
    out = np.zeros((B, OUTS[2]), np.float32)
    for core in range(NCORES):
        out += res.results[core]["out"]
    return out
